# revision 28
# baseline (speedup 1.0000x reference)
"""Trainium2 Bass kernel for nn_MultiHeadAttention_79534204387726.

Reference computation (B=4, S=1024, E=1024, H=16, dh=64):
    q/k/v = proj(x) = x @ Wq_w.T + Wq_b       (same Wq applied to q, k, v)
    scores = q @ k.T / 8 per head; attn = softmax(scores)
    out = (attn @ v).concat_heads @ Wo_w.T + Wo_b

Sharding (8 cores): core c -> batch b = c//2, head-group g = c%2 (8 heads,
512 features). Each core computes its head-group's attention output C_g
[S, 512] and the PARTIAL output projection C_g @ Wo[:, g-half].T -> [S, E].
The host sums the two partials per batch (the "all-reduce after Wo" done
host-side) and adds a folded bias.

Math simplifications (exact):
  - K-bias is softmax-invariant (constant shift along the key axis) -> dropped.
  - V-bias passes through softmax unchanged (attn rows sum to 1), so its
    contribution is (Wo_w @ Wq_b); folded into the host-side bias with Wo_b.
  - Softmax computed without max-subtraction: scores are tightly bounded
    (|s| < ~3), exp is safe in fp32. The denominator is built by folding the
    8 key-tiles of exp(scores^T) on DVE (fp16 2x mode) and finishing with a
    gpsimd partition all-reduce, which also broadcasts 1/l to all partitions.

Layouts keep everything transposed so softmax's key-axis reduction lands on
the matmul contraction (partition) axis and no on-chip transposes are needed:
  QT/KT [j, s] -> scoresT [k, q] -> exp -> PT -> AV gives OT=C^T [d, q]
  -> out-proj uses C^T tiles as stationary operands -> out [s, o] natural.

On-chip dtypes: fp16 matmul inputs (full PE rate like bf16 but 4x the
mantissa; fp32 matmul is half-rate), fp32 PSUM accumulation everywhere,
fp32 output. Heads are processed in pairs living at partitions 0-63 /
64-127 of one j-tile: score matmuls (K=dh=64) issue back-to-back on
disjoint PE row-groups and overlap, and the AV matmuls are col-tiled
(tile_position 0/64) so the pair shares one 128-partition psum tile.
"""

import numpy as np
import ml_dtypes

B, S, E, H = 4, 1024, 1024, 16
NCORES = 8
EH = E // 2        # 512 features per head-group
NHG = H // 2       # 8 heads per group
DH = E // H        # 64
P = 128
NE = E // P        # 8 e-tiles over full E
NJ = EH // P       # 4 j-tiles over the group's 512 features
NQ = S // 512      # 2 query/sequence chunks of 512
NST = S // P       # 8 sequence tiles of 128
BF16 = np.float16

_CACHE = {}


def _build_program(reps=1, av_mode="packed"):
    import concourse.tile as tile
    from concourse import bacc, mybir
    from contextlib import ExitStack

    f32 = mybir.dt.float32
    bf16 = mybir.dt.float16
    AF = mybir.ActivationFunctionType

    nc = bacc.Bacc(
        "TRN2",
        target_bir_lowering=False,
        debug=False,
        num_devices=NCORES,
    )

    xq_t = nc.dram_tensor("xq_t", [E, S], bf16, kind="ExternalInput")
    xk_t = nc.dram_tensor("xk_t", [E, S], bf16, kind="ExternalInput")
    xv_t = nc.dram_tensor("xv_t", [E, S], bf16, kind="ExternalInput")
    wq_t = nc.dram_tensor("wq_t", [E, EH], bf16, kind="ExternalInput")
    wo_t = nc.dram_tensor("wo_t", [EH, E], bf16, kind="ExternalInput")
    bq = nc.dram_tensor("bq", [P, NJ], f32, kind="ExternalInput")
    out_d = nc.dram_tensor("out_partial", [S, E], f32, kind="ExternalOutput")

    with tile.TileContext(nc) as tc, ExitStack() as ctx:
        const = ctx.enter_context(tc.tile_pool(name="const", bufs=1))
        pt_pool = ctx.enter_context(tc.tile_pool(name="pt", bufs=4))
        fold_pool = ctx.enter_context(tc.tile_pool(name="fold", bufs=2))
        rl_pool = ctx.enter_context(tc.tile_pool(name="rl", bufs=2))
        outp = ctx.enter_context(tc.tile_pool(name="outp", bufs=3))
        ps_pool = ctx.enter_context(tc.tile_pool(name="ps", bufs=2, space="PSUM"))
        ps_s = ctx.enter_context(tc.tile_pool(name="ps_s", bufs=4, space="PSUM"))
        ps_o = ctx.enter_context(tc.tile_pool(name="ps_o", bufs=2, space="PSUM"))

        # ---- resident SBUF tensors (separate tiles per j/s-tile so the
        # scheduler's dependencies stay fine-grained) ----
        wq_sb = [const.tile([P, EH], bf16, tag=f"wq{t}", name=f"wq{t}")
                 for t in range(NE)]                     # per e-tile
        wo_sb = const.tile([P, NJ, E], bf16)             # [p, e4-tile, o]
        bq_sb = const.tile([P, NJ], f32)
        xq_sb = [const.tile([P, S], bf16, tag=f"xq{t}", name=f"xq{t}") for t in range(NE)]
        xk_sb = [const.tile([P, S], bf16, tag=f"xk{t}", name=f"xk{t}") for t in range(NE)]
        xv_sb = [const.tile([P, S], bf16, tag=f"xv{t}", name=f"xv{t}") for t in range(NE)]
        qt_sb = [const.tile([P, S], bf16, tag=f"qt{j}", name=f"qt{j}") for j in range(NJ)]
        kt_sb = [const.tile([P, S], bf16, tag=f"kt{j}", name=f"kt{j}") for j in range(NJ)]
        # V tiles [s-tile][p, 8 heads x dh (+ ones col in "ones" mode)]
        vw = DH + (1 if av_mode == "ones" else 0)
        v_sb = [const.tile([P, NHG * vw], bf16, tag=f"v{st}", name=f"v{st}")
                for st in range(NST)]
        c_sb = [const.tile([P, S], bf16, tag=f"c{j}", name=f"c{j}") for j in range(NJ)]

        nc.sync.dma_start(out=bq_sb[:, :], in_=bq[:, :])
        for t in range(NE):
            nc.sync.dma_start(out=wq_sb[t], in_=wq_t[t * P:(t + 1) * P, :])
        for t in range(NE):
            nc.sync.dma_start(out=xk_sb[t], in_=xk_t[t * P:(t + 1) * P, :])
        for t in range(NE):
            nc.sync.dma_start(out=xq_sb[t], in_=xq_t[t * P:(t + 1) * P, :])
        for t in range(NE):
            nc.sync.dma_start(out=xv_sb[t], in_=xv_t[t * P:(t + 1) * P, :])
        for t in range(NJ):
            nc.sync.dma_start(out=wo_sb[:, t, :], in_=wo_t[t * P:(t + 1) * P, :])

        def body():
            if av_mode == "ones":
                for st in range(NST):
                    vh = v_sb[st].rearrange("p (h c) -> p h c", c=DH + 1)
                    nc.vector.memset(vh[:, :, DH], 1.0)

            def proj_qk(jt, x_tiles, dst, bias):
                # dst[j, s] = Wq-tile.T @ x^T, j-tile jt
                for qc in range(NQ):
                    ps = ps_pool.tile([P, 512], f32, tag="ps")
                    for t in range(NE):
                        nc.tensor.matmul(
                            ps,
                            lhsT=wq_sb[t][:, jt * P:(jt + 1) * P],
                            rhs=x_tiles[t][:, qc * 512:(qc + 1) * 512],
                            start=(t == 0),
                            stop=(t == NE - 1),
                        )
                    d = dst[:, qc * 512:(qc + 1) * 512]
                    if bias is not None:
                        nc.vector.tensor_scalar_add(d, ps, bias)
                    else:
                        nc.vector.tensor_copy(d, ps)

            def proj_v(st):
                # V[s-tile, :] with ones col; strided single copy per s-tile
                ps = ps_pool.tile([P, 512], f32, tag="ps")
                for t in range(NE):
                    nc.tensor.matmul(
                        ps,
                        lhsT=xv_sb[t][:, st * P:(st + 1) * P],
                        rhs=wq_sb[t],
                        start=(t == 0),
                        stop=(t == NE - 1),
                    )
                if av_mode == "ones":
                    vh = v_sb[st].rearrange("p (h c) -> p h c", c=DH + 1)
                    nc.vector.tensor_copy(
                        vh[:, :, 0:DH], ps.rearrange("p (h d) -> p h d", d=DH))
                else:
                    nc.vector.tensor_copy(v_sb[st], ps)

            def scores_exp(jt, pt_pair):
                # score^T tiles for the head pair at j-tile jt; the two
                # heads' lhsT live at base partitions 0/64 -> adjacent MMs
                # run on disjoint PE row groups concurrently
                for kt in range(NE):
                    for qc in range(NQ):
                        pss = []
                        for hh in range(2):
                            bp = 64 * hh
                            ps = ps_s.tile([P, 512], f32, tag="ps_s")
                            pss.append(ps)
                            nc.tensor.matmul(
                                ps,
                                lhsT=kt_sb[jt][bp:bp + DH, kt * P:(kt + 1) * P],
                                rhs=qt_sb[jt][bp:bp + DH, qc * 512:(qc + 1) * 512],
                                start=True, stop=True,
                            )
                        for hh in range(2):
                            nc.scalar.activation(
                                out=pt_pair[hh][:, kt, qc * 512:(qc + 1) * 512],
                                in_=pss[hh],
                                func=AF.Exp, scale=0.125,
                            )

            def denom(jt, hh, pt):
                # softmax denominator for head h = 2*jt + hh: fold the 8
                # key-tiles of exp(scores^T) on DVE (fp16 4x mode), then a
                # gpsimd partition all-reduce gives l broadcast to all
                # partitions; reciprocal in place on this head's 64 rows
                import concourse.bass_isa as bass_isa
                bp = 64 * hh
                f = [fold_pool.tile([P, S], bf16, tag=f"fold{i}", name=f"fold{i}")
                     for i in range(4)]
                for i in range(4):
                    nc.vector.tensor_add(f[i], pt[:, 2 * i, :], pt[:, 2 * i + 1, :])
                nc.vector.tensor_add(f[0], f[0], f[1])
                nc.vector.tensor_add(f[2], f[2], f[3])
                nc.vector.tensor_add(f[0], f[0], f[2])
                rl = rl_pool.tile([P, S], f32, tag=f"rl{hh}", name=f"rl{hh}")
                nc.gpsimd.partition_all_reduce(
                    rl, f[0], channels=P, reduce_op=bass_isa.ReduceOp.add
                )
                nc.vector.reciprocal(rl[bp:bp + DH, :], rl[bp:bp + DH, :])
                return rl

            def av_ones(jt, pts):
                # M=65 AV with ones column: denominator lands in psum row DH
                for hh in range(2):
                    h = 2 * jt + hh
                    bp = 64 * hh
                    for qc in range(NQ):
                        po = ps_o.tile([P, 512], f32, tag="ps_o")
                        for kt in range(NE):
                            nc.tensor.matmul(
                                po[0:DH + 1, :],
                                lhsT=v_sb[kt][:, h * (DH + 1):(h + 1) * (DH + 1)],
                                rhs=pts[hh][:, kt, qc * 512:(qc + 1) * 512],
                                start=(kt == 0),
                                stop=(kt == NE - 1),
                            )
                        rden = rl_pool.tile([1, 512], f32, tag="rden", name="rden")
                        nc.vector.reciprocal(rden, po[DH:DH + 1, :])
                        rb = rl_pool.tile([DH, 512], f32, tag="rb", name="rb")
                        nc.gpsimd.partition_broadcast(rb, rden, channels=DH)
                        nc.vector.tensor_mul(
                            c_sb[jt][bp:bp + DH, qc * 512:(qc + 1) * 512],
                            po[0:DH, :], rb,
                        )

            def av_pair(jt, pts, rls):
                # col-tiled AV: head A on PE columns 0-63 -> psum rows 0-63,
                # head B on columns 64-127 -> psum rows 64-127
                for qc in range(NQ):
                    po = ps_o.tile([P, 512], f32, tag="ps_o")
                    for kt in range(NE):
                        for hh in range(2):
                            h = 2 * jt + hh
                            bp = 64 * hh
                            nc.tensor.matmul(
                                po[bp:bp + DH, :],
                                lhsT=v_sb[kt][:, (2 * jt + hh) * DH:(2 * jt + hh + 1) * DH],
                                rhs=pts[hh][:, kt, qc * 512:(qc + 1) * 512],
                                start=(kt == 0),
                                stop=(kt == NE - 1),
                                tile_position=(0, bp),
                            )
                    for hh in range(2):
                        bp = 64 * hh
                        nc.vector.tensor_mul(
                            c_sb[jt][bp:bp + DH, qc * 512:(qc + 1) * 512],
                            po[bp:bp + DH, :],
                            rls[hh][bp:bp + DH, qc * 512:(qc + 1) * 512],
                        )

            def out_proj(st, oc):
                ps = ps_pool.tile([P, 512], f32, tag="ps")
                for et in range(NJ):
                    nc.tensor.matmul(
                        ps,
                        lhsT=c_sb[et][:, st * P:(st + 1) * P],
                        rhs=wo_sb[:, et, oc * 512:(oc + 1) * 512],
                        start=(et == 0),
                        stop=(et == NJ - 1),
                    )
                ot = outp.tile([P, 512], f32, tag="ot")
                nc.scalar.copy(ot, ps)
                nc.sync.dma_start(
                    out=out_d[st * P:(st + 1) * P, oc * 512:(oc + 1) * 512],
                    in_=ot,
                )

            # ---- emission order: interleave so V-proj / next j-tile's
            # projections (PE work) can fill the ACT-bound exp stretches ----
            # pipeline: pair 0's scores first, then all of V, then each
            # subsequent pair's scores followed by the previous pair's AV --
            # so pt/rl pool slots (bufs=2) recycle without stalling the flow
            pt_pairs = []
            rl_pairs = []

            def scores_block(jt):
                proj_qk(jt, xk_sb, kt_sb[jt], None)
                proj_qk(jt, xq_sb, qt_sb[jt], bq_sb[:, jt:jt + 1])
                pair = [pt_pool.tile([P, NE, S], bf16, tag="pt", name=f"pt{hh}")
                        for hh in range(2)]
                pt_pairs.append(pair)
                scores_exp(jt, pair)
                if av_mode == "ones":
                    rl_pairs.append(None)
                else:
                    rl_pairs.append([denom(jt, hh, pair[hh]) for hh in range(2)])

            scores_block(0)
            for st in range(NST):
                proj_v(st)
            def do_av(jt):
                if av_mode == "ones":
                    av_ones(jt, pt_pairs[jt])
                else:
                    av_pair(jt, pt_pairs[jt], rl_pairs[jt])

            for jt in range(1, NJ):
                scores_block(jt)
                do_av(jt - 1)
            do_av(NJ - 1)
            for st in range(NST):
                for oc in range(NQ):
                    out_proj(st, oc)

        for _ in range(reps):
            body()

    nc.finalize()
    return nc


def _get_nc(reps=1, av_mode="packed"):
    key = ("nc", reps, av_mode)
    if key not in _CACHE:
        _CACHE[key] = _build_program(reps, av_mode)
    return _CACHE[key]


def make_in_maps(queries, keys, values, Wq_w, Wq_b, Wo_w, Wo_b):
    in_maps = []
    for c in range(NCORES):
        b, g = c // 2, c % 2
        js = slice(g * EH, (g + 1) * EH)
        in_maps.append({
            "xq_t": np.ascontiguousarray(queries[b].T).astype(BF16),
            "xk_t": np.ascontiguousarray(keys[b].T).astype(BF16),
            "xv_t": np.ascontiguousarray(values[b].T).astype(BF16),
            "wq_t": np.ascontiguousarray(Wq_w[js, :].T).astype(BF16),
            "wo_t": np.ascontiguousarray(Wo_w[:, js].T).astype(BF16),
            "bq": np.ascontiguousarray(Wq_b[js].reshape(NJ, P).T),
        })
    return in_maps


def assemble_output(results, Wq_b, Wo_w, Wo_b):
    # host-side unshard: sum the two head-group partials per batch, add the
    # folded bias (Wo_b + V-bias routed through Wo since attn rows sum to 1)
    bias_total = (Wo_w @ Wq_b + Wo_b).astype(np.float32)
    out = np.empty((B, S, E), np.float32)
    for b in range(B):
        out[b] = results[2 * b]["out_partial"] + results[2 * b + 1]["out_partial"]
    out += bias_total
    return out


def kernel(queries, keys, values, Wq_w, Wq_b, Wo_w, Wo_b, num_heads):
    from concourse.bass_utils import run_bass_kernel_spmd

    queries = np.asarray(queries, np.float32)
    keys = np.asarray(keys, np.float32)
    values = np.asarray(values, np.float32)
    Wq_w = np.asarray(Wq_w, np.float32)
    Wq_b = np.asarray(Wq_b, np.float32)
    Wo_w = np.asarray(Wo_w, np.float32)
    Wo_b = np.asarray(Wo_b, np.float32)
    assert int(num_heads) == H

    nc = _get_nc()
    in_maps = make_in_maps(queries, keys, values, Wq_w, Wq_b, Wo_w, Wo_b)
    res = run_bass_kernel_spmd(nc, in_maps, core_ids=list(range(NCORES)))
    _CACHE["last_results"] = res
    return assemble_output(res.results, Wq_b, Wo_w, Wo_b)


# revision 32
# speedup vs baseline: 1.9744x; 1.9744x over previous
"""Trainium2 Bass kernel for nn_MultiHeadAttention_79534204387726.

Reference computation (B=4, S=1024, E=1024, H=16, dh=64):
    q/k/v = proj(x) = x @ Wq_w.T + Wq_b       (same Wq applied to q, k, v)
    scores = q @ k.T / 8 per head; attn = softmax(scores)
    out = (attn @ v).concat_heads @ Wo_w.T + Wo_b

Sharding (8 cores): core c -> batch b = c//2, head-group g = c%2 (8 heads,
512 features). Each core computes its head-group's attention output C_g
[S, 512] and the PARTIAL output projection C_g @ Wo[:, g-half].T -> [S, E].
The host sums the two partials per batch (the "all-reduce after Wo" done
host-side) and adds a folded bias.

Math simplifications (exact):
  - K-bias is softmax-invariant (constant shift along the key axis) -> dropped.
  - V-bias passes through softmax unchanged (attn rows sum to 1), so its
    contribution is (Wo_w @ Wq_b); folded into the host-side bias with Wo_b.
  - Softmax computed without max-subtraction: scores are tightly bounded
    (|s| < ~3), exp is safe in fp32. The denominator is built by folding the
    8 key-tiles of exp(scores^T) on DVE (fp16 2x mode) and finishing with a
    gpsimd partition all-reduce, which also broadcasts 1/l to all partitions.

Layouts keep everything transposed so softmax's key-axis reduction lands on
the matmul contraction (partition) axis and no on-chip transposes are needed:
  QT/KT [j, s] -> scoresT [k, q] -> exp -> PT -> AV gives OT=C^T [d, q]
  -> out-proj uses C^T tiles as stationary operands -> out [s, o] natural.

On-chip dtypes: fp16 matmul inputs (full PE rate like bf16 but 4x the
mantissa; fp32 matmul is half-rate), fp32 PSUM accumulation everywhere,
fp32 output. Heads are processed in pairs living at partitions 0-63 /
64-127 of one j-tile: score matmuls (K=dh=64) issue back-to-back on
disjoint PE row-groups and overlap, and the AV matmuls are col-tiled
(tile_position 0/64) so the pair shares one 128-partition psum tile.
"""

import numpy as np
import ml_dtypes

B, S, E, H = 4, 1024, 1024, 16
NCORES = 8
EH = E // 2        # 512 features per head-group
NHG = H // 2       # 8 heads per group
DH = E // H        # 64
P = 128
NE = E // P        # 8 e-tiles over full E
NJ = EH // P       # 4 j-tiles over the group's 512 features
NQ = S // 512      # 2 query/sequence chunks of 512
NST = S // P       # 8 sequence tiles of 128
BF16 = np.float16

_CACHE = {}


def _build_program(reps=1, av_mode="packed"):
    import concourse.tile as tile
    from concourse import bacc, mybir
    from contextlib import ExitStack

    f32 = mybir.dt.float32
    bf16 = mybir.dt.float16
    AF = mybir.ActivationFunctionType

    nc = bacc.Bacc(
        "TRN2",
        target_bir_lowering=False,
        debug=False,
        num_devices=NCORES,
    )

    xq_t = nc.dram_tensor("xq_t", [E, S], bf16, kind="ExternalInput")
    xk_t = nc.dram_tensor("xk_t", [E, S], bf16, kind="ExternalInput")
    xv_t = nc.dram_tensor("xv_t", [E, S], bf16, kind="ExternalInput")
    wq_t = nc.dram_tensor("wq_t", [E, EH], bf16, kind="ExternalInput")
    wo_t = nc.dram_tensor("wo_t", [EH, E], bf16, kind="ExternalInput")
    bq = nc.dram_tensor("bq", [P, NJ], f32, kind="ExternalInput")
    out_d = nc.dram_tensor("out_partial", [S, E], f32, kind="ExternalOutput")

    with tile.TileContext(nc) as tc, ExitStack() as ctx:
        const = ctx.enter_context(tc.tile_pool(name="const", bufs=1))
        pt_pool = ctx.enter_context(tc.tile_pool(name="pt", bufs=4))
        fold_pool = ctx.enter_context(tc.tile_pool(name="fold", bufs=2))
        rl_pool = ctx.enter_context(tc.tile_pool(name="rl", bufs=2))
        outp = ctx.enter_context(tc.tile_pool(name="outp", bufs=3))
        ps_pool = ctx.enter_context(tc.tile_pool(name="ps", bufs=2, space="PSUM"))
        ps_s = ctx.enter_context(tc.tile_pool(name="ps_s", bufs=4, space="PSUM"))
        ps_o = ctx.enter_context(tc.tile_pool(name="ps_o", bufs=2, space="PSUM"))

        # ---- resident SBUF tensors (separate tiles per j/s-tile so the
        # scheduler's dependencies stay fine-grained) ----
        wq_sb = [const.tile([P, EH], bf16, tag=f"wq{t}", name=f"wq{t}")
                 for t in range(NE)]                     # per e-tile
        wo_sb = const.tile([P, NJ, E], bf16)             # [p, e4-tile, o]
        bq_sb = const.tile([P, NJ], f32)
        xq_sb = [const.tile([P, S], bf16, tag=f"xq{t}", name=f"xq{t}") for t in range(NE)]
        xk_sb = [const.tile([P, S], bf16, tag=f"xk{t}", name=f"xk{t}") for t in range(NE)]
        xv_sb = [const.tile([P, S], bf16, tag=f"xv{t}", name=f"xv{t}") for t in range(NE)]
        qt_sb = [const.tile([P, S], bf16, tag=f"qt{j}", name=f"qt{j}") for j in range(NJ)]
        kt_sb = [const.tile([P, S], bf16, tag=f"kt{j}", name=f"kt{j}") for j in range(NJ)]
        # V tiles [s-tile][p, 8 heads x dh (+ ones col in "ones" mode)]
        vw = DH + (1 if av_mode == "ones" else 0)
        v_sb = [const.tile([P, NHG * vw], bf16, tag=f"v{st}", name=f"v{st}")
                for st in range(NST)]
        c_sb = [const.tile([P, S], bf16, tag=f"c{j}", name=f"c{j}") for j in range(NJ)]

        nc.sync.dma_start(out=bq_sb[:, :], in_=bq[:, :])
        for t in range(NE):
            nc.sync.dma_start(out=wq_sb[t], in_=wq_t[t * P:(t + 1) * P, :])
            nc.sync.dma_start(out=xk_sb[t], in_=xk_t[t * P:(t + 1) * P, :])
        for t in range(NE):
            nc.sync.dma_start(out=xq_sb[t], in_=xq_t[t * P:(t + 1) * P, :])
        for t in range(NE):
            nc.sync.dma_start(out=xv_sb[t], in_=xv_t[t * P:(t + 1) * P, :])
        for t in range(NJ):
            nc.sync.dma_start(out=wo_sb[:, t, :], in_=wo_t[t * P:(t + 1) * P, :])

        def body():
            if av_mode == "ones":
                for st in range(NST):
                    vh = v_sb[st].rearrange("p (h c) -> p h c", c=DH + 1)
                    nc.vector.memset(vh[:, :, DH], 1.0)

            def proj_qk(jt, x_tiles, dst, bias):
                # dst[j, s] = Wq-tile.T @ x^T, j-tile jt
                for qc in range(NQ):
                    ps = ps_pool.tile([P, 512], f32, tag="ps")
                    for t in range(NE):
                        nc.tensor.matmul(
                            ps,
                            lhsT=wq_sb[t][:, jt * P:(jt + 1) * P],
                            rhs=x_tiles[t][:, qc * 512:(qc + 1) * 512],
                            start=(t == 0),
                            stop=(t == NE - 1),
                        )
                    d = dst[:, qc * 512:(qc + 1) * 512]
                    if bias is not None:
                        nc.vector.tensor_scalar_add(d, ps, bias)
                    else:
                        nc.vector.tensor_copy(d, ps)

            def proj_v(st):
                # V[s-tile, :] with ones col; strided single copy per s-tile
                ps = ps_pool.tile([P, 512], f32, tag="ps")
                for t in range(NE):
                    nc.tensor.matmul(
                        ps,
                        lhsT=xv_sb[t][:, st * P:(st + 1) * P],
                        rhs=wq_sb[t],
                        start=(t == 0),
                        stop=(t == NE - 1),
                    )
                if av_mode == "ones":
                    vh = v_sb[st].rearrange("p (h c) -> p h c", c=DH + 1)
                    nc.vector.tensor_copy(
                        vh[:, :, 0:DH], ps.rearrange("p (h d) -> p h d", d=DH))
                else:
                    nc.vector.tensor_copy(v_sb[st], ps)

            def scores_exp(jt, pt_pair):
                # score^T tiles for the head pair at j-tile jt; the two
                # heads' lhsT live at base partitions 0/64 -> adjacent MMs
                # run on disjoint PE row groups concurrently
                for kt in range(NE):
                    for qc in range(NQ):
                        pss = []
                        for hh in range(2):
                            bp = 64 * hh
                            ps = ps_s.tile([P, 512], f32, tag="ps_s")
                            pss.append(ps)
                            nc.tensor.matmul(
                                ps,
                                lhsT=kt_sb[jt][bp:bp + DH, kt * P:(kt + 1) * P],
                                rhs=qt_sb[jt][bp:bp + DH, qc * 512:(qc + 1) * 512],
                                start=True, stop=True,
                            )
                        for hh in range(2):
                            nc.scalar.activation(
                                out=pt_pair[hh][:, kt, qc * 512:(qc + 1) * 512],
                                in_=pss[hh],
                                func=AF.Exp, scale=0.125,
                            )

            def denom_qc(jt, hh, pt):
                # per-q-chunk denominator (used for the last pair): lets the
                # qc0 half of c finish early so out-proj st<4 groups unblock
                import concourse.bass_isa as bass_isa
                bp = 64 * hh
                rls = []
                for qc in range(NQ):
                    sl = slice(qc * 512, (qc + 1) * 512)
                    f = [fold_pool.tile([P, 512], bf16, tag=f"fold{i}",
                                        name=f"fq{i}") for i in range(4)]
                    for i in range(4):
                        nc.vector.tensor_add(
                            f[i], pt[:, 2 * i, sl], pt[:, 2 * i + 1, sl])
                    nc.vector.tensor_add(f[0], f[0], f[1])
                    nc.vector.tensor_add(f[2], f[2], f[3])
                    nc.vector.tensor_add(f[0], f[0], f[2])
                    rl = rl_pool.tile([P, 512], f32, tag=f"rq{hh}{qc}",
                                      name=f"rq{hh}{qc}", bufs=1)
                    nc.gpsimd.partition_all_reduce(
                        rl, f[0], channels=P, reduce_op=bass_isa.ReduceOp.add
                    )
                    nc.vector.reciprocal(rl[bp:bp + DH, :], rl[bp:bp + DH, :])
                    rls.append(rl)
                return rls

            def denom(jt, hh, pt):
                # softmax denominator for head h = 2*jt + hh: fold the 8
                # key-tiles of exp(scores^T) on DVE (fp16 4x mode), then a
                # gpsimd partition all-reduce gives l broadcast to all
                # partitions; reciprocal in place on this head's 64 rows
                import concourse.bass_isa as bass_isa
                bp = 64 * hh
                f = [fold_pool.tile([P, S], bf16, tag=f"fold{i}", name=f"fold{i}")
                     for i in range(4)]
                for i in range(4):
                    nc.vector.tensor_add(f[i], pt[:, 2 * i, :], pt[:, 2 * i + 1, :])
                nc.vector.tensor_add(f[0], f[0], f[1])
                nc.vector.tensor_add(f[2], f[2], f[3])
                nc.vector.tensor_add(f[0], f[0], f[2])
                rl = rl_pool.tile([P, S], f32, tag=f"rl{hh}", name=f"rl{hh}",
                                  bufs=1)
                nc.gpsimd.partition_all_reduce(
                    rl, f[0], channels=P, reduce_op=bass_isa.ReduceOp.add
                )
                nc.vector.reciprocal(rl[bp:bp + DH, :], rl[bp:bp + DH, :])
                return rl

            def av_ones(jt, pts):
                # M=65 AV with ones column: denominator lands in psum row DH
                for hh in range(2):
                    h = 2 * jt + hh
                    bp = 64 * hh
                    for qc in range(NQ):
                        po = ps_o.tile([P, 512], f32, tag="ps_o")
                        for kt in range(NE):
                            nc.tensor.matmul(
                                po[0:DH + 1, :],
                                lhsT=v_sb[kt][:, h * (DH + 1):(h + 1) * (DH + 1)],
                                rhs=pts[hh][:, kt, qc * 512:(qc + 1) * 512],
                                start=(kt == 0),
                                stop=(kt == NE - 1),
                            )
                        rden = rl_pool.tile([1, 512], f32, tag="rden", name="rden")
                        nc.vector.reciprocal(rden, po[DH:DH + 1, :])
                        rb = rl_pool.tile([DH, 512], f32, tag="rb", name="rb")
                        nc.gpsimd.partition_broadcast(rb, rden, channels=DH)
                        nc.vector.tensor_mul(
                            c_sb[jt][bp:bp + DH, qc * 512:(qc + 1) * 512],
                            po[0:DH, :], rb,
                        )

            def av_pair(jt, pts, rls):
                # col-tiled AV: head A on PE columns 0-63 -> psum rows 0-63,
                # head B on columns 64-127 -> psum rows 64-127
                for qc in range(NQ):
                    po = ps_o.tile([P, 512], f32, tag="ps_o")
                    for kt in range(NE):
                        for hh in range(2):
                            h = 2 * jt + hh
                            bp = 64 * hh
                            nc.tensor.matmul(
                                po[bp:bp + DH, :],
                                lhsT=v_sb[kt][:, (2 * jt + hh) * DH:(2 * jt + hh + 1) * DH],
                                rhs=pts[hh][:, kt, qc * 512:(qc + 1) * 512],
                                start=(kt == 0),
                                stop=(kt == NE - 1),
                                tile_position=(0, bp),
                            )
                    for hh in range(2):
                        bp = 64 * hh
                        r = rls[hh]
                        rsl = (r[qc][bp:bp + DH, :] if isinstance(r, list)
                               else r[bp:bp + DH, qc * 512:(qc + 1) * 512])
                        nc.vector.tensor_mul(
                            c_sb[jt][bp:bp + DH, qc * 512:(qc + 1) * 512],
                            po[bp:bp + DH, :], rsl,
                        )

            def out_proj(st, oc):
                ps = ps_pool.tile([P, 512], f32, tag="ps")
                for et in range(NJ):
                    nc.tensor.matmul(
                        ps,
                        lhsT=c_sb[et][:, st * P:(st + 1) * P],
                        rhs=wo_sb[:, et, oc * 512:(oc + 1) * 512],
                        start=(et == 0),
                        stop=(et == NJ - 1),
                    )
                ot = outp.tile([P, 512], f32, tag="ot")
                nc.scalar.copy(ot, ps)
                nc.sync.dma_start(
                    out=out_d[st * P:(st + 1) * P, oc * 512:(oc + 1) * 512],
                    in_=ot,
                )

            # ---- emission order: interleave so V-proj / next j-tile's
            # projections (PE work) can fill the ACT-bound exp stretches ----
            # pipeline: pair 0's scores first, then all of V, then each
            # subsequent pair's scores followed by the previous pair's AV --
            # so pt/rl pool slots (bufs=2) recycle without stalling the flow
            pt_pairs = []
            rl_pairs = []

            def scores_block(jt):
                proj_qk(jt, xk_sb, kt_sb[jt], None)
                proj_qk(jt, xq_sb, qt_sb[jt], bq_sb[:, jt:jt + 1])
                pair = [pt_pool.tile([P, NE, S], bf16, tag="pt", name=f"pt{hh}")
                        for hh in range(2)]
                pt_pairs.append(pair)
                scores_exp(jt, pair)
                if av_mode == "ones":
                    rl_pairs.append(None)
                elif jt == NJ - 1:
                    rl_pairs.append(
                        [denom_qc(jt, hh, pair[hh]) for hh in range(2)])
                else:
                    rl_pairs.append([denom(jt, hh, pair[hh]) for hh in range(2)])

            scores_block(0)
            for st in range(NST):
                proj_v(st)
            def do_av(jt):
                if av_mode == "ones":
                    av_ones(jt, pt_pairs[jt])
                else:
                    av_pair(jt, pt_pairs[jt], rl_pairs[jt])

            for jt in range(1, NJ):
                scores_block(jt)
                do_av(jt - 1)
            do_av(NJ - 1)
            for st in range(NST):
                for oc in range(NQ):
                    out_proj(st, oc)

        for _ in range(reps):
            body()

    nc.finalize()
    return nc


def _get_nc(reps=1, av_mode="packed"):
    key = ("nc", reps, av_mode)
    if key not in _CACHE:
        _CACHE[key] = _build_program(reps, av_mode)
    return _CACHE[key]


def make_in_maps(queries, keys, values, Wq_w, Wq_b, Wo_w, Wo_b):
    in_maps = []
    for c in range(NCORES):
        b, g = c // 2, c % 2
        js = slice(g * EH, (g + 1) * EH)
        in_maps.append({
            "xq_t": np.ascontiguousarray(queries[b].T).astype(BF16),
            "xk_t": np.ascontiguousarray(keys[b].T).astype(BF16),
            "xv_t": np.ascontiguousarray(values[b].T).astype(BF16),
            "wq_t": np.ascontiguousarray(Wq_w[js, :].T).astype(BF16),
            "wo_t": np.ascontiguousarray(Wo_w[:, js].T).astype(BF16),
            "bq": np.ascontiguousarray(Wq_b[js].reshape(NJ, P).T),
        })
    return in_maps


def assemble_output(results, Wq_b, Wo_w, Wo_b):
    # host-side unshard: sum the two head-group partials per batch, add the
    # folded bias (Wo_b + V-bias routed through Wo since attn rows sum to 1)
    bias_total = (Wo_w @ Wq_b + Wo_b).astype(np.float32)
    out = np.empty((B, S, E), np.float32)
    for b in range(B):
        out[b] = results[2 * b]["out_partial"] + results[2 * b + 1]["out_partial"]
    out += bias_total
    return out


def kernel(queries, keys, values, Wq_w, Wq_b, Wo_w, Wo_b, num_heads):
    from concourse.bass_utils import run_bass_kernel_spmd

    queries = np.asarray(queries, np.float32)
    keys = np.asarray(keys, np.float32)
    values = np.asarray(values, np.float32)
    Wq_w = np.asarray(Wq_w, np.float32)
    Wq_b = np.asarray(Wq_b, np.float32)
    Wo_w = np.asarray(Wo_w, np.float32)
    Wo_b = np.asarray(Wo_b, np.float32)
    assert int(num_heads) == H

    nc = _get_nc()
    in_maps = make_in_maps(queries, keys, values, Wq_w, Wq_b, Wo_w, Wo_b)
    res = run_bass_kernel_spmd(nc, in_maps, core_ids=list(range(NCORES)))
    _CACHE["last_results"] = res
    return assemble_output(res.results, Wq_b, Wo_w, Wo_b)


# revision 33
# speedup vs baseline: 2.0595x; 1.0431x over previous
"""Trainium2 Bass kernel for nn_MultiHeadAttention_79534204387726.

Reference computation (B=4, S=1024, E=1024, H=16, dh=64):
    q/k/v = proj(x) = x @ Wq_w.T + Wq_b       (same Wq applied to q, k, v)
    scores = q @ k.T / 8 per head; attn = softmax(scores)
    out = (attn @ v).concat_heads @ Wo_w.T + Wo_b

Sharding (8 cores): core c -> batch b = c//2, head-group g = c%2 (8 heads,
512 features). Each core computes its head-group's attention output C_g
[S, 512] and the PARTIAL output projection C_g @ Wo[:, g-half].T -> [S, E].
The host sums the two partials per batch (the "all-reduce after Wo" done
host-side) and adds a folded bias.

Math simplifications (exact):
  - K-bias is softmax-invariant (constant shift along the key axis) -> dropped.
  - V-bias passes through softmax unchanged (attn rows sum to 1), so its
    contribution is (Wo_w @ Wq_b); folded into the host-side bias with Wo_b.
  - Softmax computed without max-subtraction: scores are tightly bounded
    (|s| < ~3), exp is safe in fp32. The denominator is built by folding the
    8 key-tiles of exp(scores^T) on DVE (fp16 2x mode) and finishing with a
    gpsimd partition all-reduce, which also broadcasts 1/l to all partitions.

Layouts keep everything transposed so softmax's key-axis reduction lands on
the matmul contraction (partition) axis and no on-chip transposes are needed:
  QT/KT [j, s] -> scoresT [k, q] -> exp -> PT -> AV gives OT=C^T [d, q]
  -> out-proj uses C^T tiles as stationary operands -> out [s, o] natural.

On-chip dtypes: fp16 matmul inputs (full PE rate like bf16 but 4x the
mantissa; fp32 matmul is half-rate), fp32 PSUM accumulation everywhere,
fp32 output. Heads are processed in pairs living at partitions 0-63 /
64-127 of one j-tile: score matmuls (K=dh=64) issue back-to-back on
disjoint PE row-groups and overlap, and the AV matmuls are col-tiled
(tile_position 0/64) so the pair shares one 128-partition psum tile.
"""

import numpy as np
import ml_dtypes

B, S, E, H = 4, 1024, 1024, 16
NCORES = 8
EH = E // 2        # 512 features per head-group
NHG = H // 2       # 8 heads per group
DH = E // H        # 64
P = 128
NE = E // P        # 8 e-tiles over full E
NJ = EH // P       # 4 j-tiles over the group's 512 features
NQ = S // 512      # 2 query/sequence chunks of 512
NST = S // P       # 8 sequence tiles of 128
BF16 = np.float16

_CACHE = {}


def _build_program(reps=1, av_mode="packed"):
    import concourse.tile as tile
    from concourse import bacc, mybir
    from contextlib import ExitStack

    f32 = mybir.dt.float32
    bf16 = mybir.dt.float16
    AF = mybir.ActivationFunctionType

    nc = bacc.Bacc(
        "TRN2",
        target_bir_lowering=False,
        debug=False,
        num_devices=NCORES,
    )

    xq_t = nc.dram_tensor("xq_t", [E, S], bf16, kind="ExternalInput")
    xk_t = nc.dram_tensor("xk_t", [E, S], bf16, kind="ExternalInput")
    xv_t = nc.dram_tensor("xv_t", [E, S], bf16, kind="ExternalInput")
    wq_t = nc.dram_tensor("wq_t", [E, EH], bf16, kind="ExternalInput")
    wo_t = nc.dram_tensor("wo_t", [EH, E], bf16, kind="ExternalInput")
    bq = nc.dram_tensor("bq", [P, NJ], f32, kind="ExternalInput")
    out_d = nc.dram_tensor("out_partial", [S, E], f32, kind="ExternalOutput")

    with tile.TileContext(nc) as tc, ExitStack() as ctx:
        const = ctx.enter_context(tc.tile_pool(name="const", bufs=1))
        pt_pool = ctx.enter_context(tc.tile_pool(name="pt", bufs=4))
        fold_pool = ctx.enter_context(tc.tile_pool(name="fold", bufs=2))
        rl_pool = ctx.enter_context(tc.tile_pool(name="rl", bufs=2))
        outp = ctx.enter_context(tc.tile_pool(name="outp", bufs=3))
        ps_pool = ctx.enter_context(tc.tile_pool(name="ps", bufs=2, space="PSUM"))
        ps_s = ctx.enter_context(tc.tile_pool(name="ps_s", bufs=4, space="PSUM"))
        ps_o = ctx.enter_context(tc.tile_pool(name="ps_o", bufs=2, space="PSUM"))

        # ---- resident SBUF tensors (separate tiles per j/s-tile so the
        # scheduler's dependencies stay fine-grained) ----
        wq_sb = [const.tile([P, EH], bf16, tag=f"wq{t}", name=f"wq{t}")
                 for t in range(NE)]                     # per e-tile
        wo_sb = const.tile([P, NJ, E], bf16)             # [p, e4-tile, o]
        bq_sb = const.tile([P, NJ], f32)
        xq_sb = [const.tile([P, S], bf16, tag=f"xq{t}", name=f"xq{t}") for t in range(NE)]
        xk_sb = [const.tile([P, S], bf16, tag=f"xk{t}", name=f"xk{t}") for t in range(NE)]
        xv_sb = [const.tile([P, S], bf16, tag=f"xv{t}", name=f"xv{t}") for t in range(NE)]
        qt_sb = [const.tile([P, S], bf16, tag=f"qt{j}", name=f"qt{j}") for j in range(NJ)]
        kt_sb = [const.tile([P, S], bf16, tag=f"kt{j}", name=f"kt{j}") for j in range(NJ)]
        # V tiles [s-tile][p, 8 heads x dh (+ ones col in "ones" mode)]
        vw = DH + (1 if av_mode == "ones" else 0)
        v_sb = [const.tile([P, NHG * vw], bf16, tag=f"v{st}", name=f"v{st}")
                for st in range(NST)]
        c_sb = [const.tile([P, S], bf16, tag=f"c{j}", name=f"c{j}") for j in range(NJ)]

        nc.sync.dma_start(out=bq_sb[:, :], in_=bq[:, :])
        for t in range(NE):
            nc.sync.dma_start(out=wq_sb[t], in_=wq_t[t * P:(t + 1) * P, :])
            nc.sync.dma_start(out=xk_sb[t], in_=xk_t[t * P:(t + 1) * P, :])
        for t in range(NE):
            nc.sync.dma_start(out=xq_sb[t], in_=xq_t[t * P:(t + 1) * P, :])
        for t in range(NE):
            nc.sync.dma_start(out=xv_sb[t], in_=xv_t[t * P:(t + 1) * P, :])
        for t in range(NJ):
            nc.sync.dma_start(out=wo_sb[:, t, :], in_=wo_t[t * P:(t + 1) * P, :])

        def body():
            if av_mode == "ones":
                for st in range(NST):
                    vh = v_sb[st].rearrange("p (h c) -> p h c", c=DH + 1)
                    nc.vector.memset(vh[:, :, DH], 1.0)

            def proj_qk(jt, x_tiles, dst, bias):
                # dst[j, s] = Wq-tile.T @ x^T, j-tile jt
                for qc in range(NQ):
                    ps = ps_pool.tile([P, 512], f32, tag="ps")
                    for t in range(NE):
                        nc.tensor.matmul(
                            ps,
                            lhsT=wq_sb[t][:, jt * P:(jt + 1) * P],
                            rhs=x_tiles[t][:, qc * 512:(qc + 1) * 512],
                            start=(t == 0),
                            stop=(t == NE - 1),
                        )
                    d = dst[:, qc * 512:(qc + 1) * 512]
                    if bias is not None:
                        nc.vector.tensor_scalar_add(d, ps, bias)
                    else:
                        nc.vector.tensor_copy(d, ps)

            def proj_v(st):
                # V[s-tile, :] with ones col; strided single copy per s-tile
                ps = ps_pool.tile([P, 512], f32, tag="ps")
                for t in range(NE):
                    nc.tensor.matmul(
                        ps,
                        lhsT=xv_sb[t][:, st * P:(st + 1) * P],
                        rhs=wq_sb[t],
                        start=(t == 0),
                        stop=(t == NE - 1),
                    )
                if av_mode == "ones":
                    vh = v_sb[st].rearrange("p (h c) -> p h c", c=DH + 1)
                    nc.vector.tensor_copy(
                        vh[:, :, 0:DH], ps.rearrange("p (h d) -> p h d", d=DH))
                else:
                    nc.vector.tensor_copy(v_sb[st], ps)

            def scores_exp(jt, pt_pair):
                # score^T tiles for the head pair at j-tile jt; the two
                # heads' lhsT live at base partitions 0/64 -> adjacent MMs
                # run on disjoint PE row groups concurrently
                for kt in range(NE):
                    for qc in range(NQ):
                        pss = []
                        for hh in range(2):
                            bp = 64 * hh
                            ps = ps_s.tile([P, 512], f32, tag="ps_s")
                            pss.append(ps)
                            nc.tensor.matmul(
                                ps,
                                lhsT=kt_sb[jt][bp:bp + DH, kt * P:(kt + 1) * P],
                                rhs=qt_sb[jt][bp:bp + DH, qc * 512:(qc + 1) * 512],
                                start=True, stop=True,
                            )
                        for hh in range(2):
                            nc.scalar.activation(
                                out=pt_pair[hh][:, kt, qc * 512:(qc + 1) * 512],
                                in_=pss[hh],
                                func=AF.Exp, scale=0.125,
                            )

            def denom_qc(jt, hh, pt):
                # per-q-chunk denominator (used for the last pair): lets the
                # qc0 half of c finish early so out-proj st<4 groups unblock
                import concourse.bass_isa as bass_isa
                bp = 64 * hh
                rls = []
                for qc in range(NQ):
                    sl = slice(qc * 512, (qc + 1) * 512)
                    f = [fold_pool.tile([P, 512], bf16, tag=f"fold{i}",
                                        name=f"fq{i}") for i in range(4)]
                    for i in range(4):
                        nc.vector.tensor_add(
                            f[i], pt[:, 2 * i, sl], pt[:, 2 * i + 1, sl])
                    nc.vector.tensor_add(f[0], f[0], f[1])
                    nc.vector.tensor_add(f[2], f[2], f[3])
                    nc.vector.tensor_add(f[0], f[0], f[2])
                    rl = rl_pool.tile([P, 512], f32, tag=f"rq{hh}{qc}",
                                      name=f"rq{hh}{qc}", bufs=1)
                    nc.gpsimd.partition_all_reduce(
                        rl, f[0], channels=P, reduce_op=bass_isa.ReduceOp.add
                    )
                    nc.vector.reciprocal(rl[bp:bp + DH, :], rl[bp:bp + DH, :])
                    rls.append(rl)
                return rls

            def denom(jt, hh, pt):
                # softmax denominator for head h = 2*jt + hh: fold the 8
                # key-tiles of exp(scores^T) on DVE (fp16 4x mode), then a
                # gpsimd partition all-reduce gives l broadcast to all
                # partitions; reciprocal in place on this head's 64 rows
                import concourse.bass_isa as bass_isa
                bp = 64 * hh
                f = [fold_pool.tile([P, S], bf16, tag=f"fold{i}", name=f"fold{i}")
                     for i in range(4)]
                for i in range(4):
                    nc.vector.tensor_add(f[i], pt[:, 2 * i, :], pt[:, 2 * i + 1, :])
                nc.vector.tensor_add(f[0], f[0], f[1])
                nc.vector.tensor_add(f[2], f[2], f[3])
                nc.vector.tensor_add(f[0], f[0], f[2])
                rl = rl_pool.tile([P, S], f32, tag=f"rl{hh}", name=f"rl{hh}",
                                  bufs=1)
                nc.gpsimd.partition_all_reduce(
                    rl, f[0], channels=P, reduce_op=bass_isa.ReduceOp.add
                )
                nc.vector.reciprocal(rl[bp:bp + DH, :], rl[bp:bp + DH, :])
                return rl

            def av_ones(jt, pts):
                # M=65 AV with ones column: denominator lands in psum row DH
                for hh in range(2):
                    h = 2 * jt + hh
                    bp = 64 * hh
                    for qc in range(NQ):
                        po = ps_o.tile([P, 512], f32, tag="ps_o")
                        for kt in range(NE):
                            nc.tensor.matmul(
                                po[0:DH + 1, :],
                                lhsT=v_sb[kt][:, h * (DH + 1):(h + 1) * (DH + 1)],
                                rhs=pts[hh][:, kt, qc * 512:(qc + 1) * 512],
                                start=(kt == 0),
                                stop=(kt == NE - 1),
                            )
                        rden = rl_pool.tile([1, 512], f32, tag="rden", name="rden")
                        nc.vector.reciprocal(rden, po[DH:DH + 1, :])
                        rb = rl_pool.tile([DH, 512], f32, tag="rb", name="rb")
                        nc.gpsimd.partition_broadcast(rb, rden, channels=DH)
                        nc.vector.tensor_mul(
                            c_sb[jt][bp:bp + DH, qc * 512:(qc + 1) * 512],
                            po[0:DH, :], rb,
                        )

            def av_last(jt, pts):
                import concourse.bass_isa as bass_isa
                for qc in range(NQ):
                    sl = slice(qc * 512, (qc + 1) * 512)
                    rls = []
                    for hh in range(2):
                        bp = 64 * hh
                        f = [fold_pool.tile([P, 512], bf16, tag=f"fold{i}",
                                            name=f"fl{i}") for i in range(4)]
                        for i in range(4):
                            nc.vector.tensor_add(
                                f[i], pts[hh][:, 2 * i, sl],
                                pts[hh][:, 2 * i + 1, sl])
                        nc.vector.tensor_add(f[0], f[0], f[1])
                        nc.vector.tensor_add(f[2], f[2], f[3])
                        nc.vector.tensor_add(f[0], f[0], f[2])
                        rl = rl_pool.tile([P, 512], f32, tag=f"rq{hh}{qc}",
                                          name=f"rq{hh}{qc}", bufs=1)
                        nc.gpsimd.partition_all_reduce(
                            rl, f[0], channels=P,
                            reduce_op=bass_isa.ReduceOp.add)
                        nc.vector.reciprocal(rl[bp:bp + DH, :], rl[bp:bp + DH, :])
                        rls.append(rl)
                    po = ps_o.tile([P, 512], f32, tag="ps_o")
                    for kt in range(NE):
                        for hh in range(2):
                            h = 2 * jt + hh
                            bp = 64 * hh
                            nc.tensor.matmul(
                                po[bp:bp + DH, :],
                                lhsT=v_sb[kt][:, h * DH:(h + 1) * DH],
                                rhs=pts[hh][:, kt, sl],
                                start=(kt == 0),
                                stop=(kt == NE - 1),
                                tile_position=(0, bp),
                            )
                    for hh in range(2):
                        bp = 64 * hh
                        nc.vector.tensor_mul(
                            c_sb[jt][bp:bp + DH, sl],
                            po[bp:bp + DH, :], rls[hh][bp:bp + DH, :],
                        )

            def av_pair(jt, pts, rls):
                # col-tiled AV: head A on PE columns 0-63 -> psum rows 0-63,
                # head B on columns 64-127 -> psum rows 64-127
                for qc in range(NQ):
                    po = ps_o.tile([P, 512], f32, tag="ps_o")
                    for kt in range(NE):
                        for hh in range(2):
                            h = 2 * jt + hh
                            bp = 64 * hh
                            nc.tensor.matmul(
                                po[bp:bp + DH, :],
                                lhsT=v_sb[kt][:, (2 * jt + hh) * DH:(2 * jt + hh + 1) * DH],
                                rhs=pts[hh][:, kt, qc * 512:(qc + 1) * 512],
                                start=(kt == 0),
                                stop=(kt == NE - 1),
                                tile_position=(0, bp),
                            )
                    for hh in range(2):
                        bp = 64 * hh
                        r = rls[hh]
                        rsl = (r[qc][bp:bp + DH, :] if isinstance(r, list)
                               else r[bp:bp + DH, qc * 512:(qc + 1) * 512])
                        nc.vector.tensor_mul(
                            c_sb[jt][bp:bp + DH, qc * 512:(qc + 1) * 512],
                            po[bp:bp + DH, :], rsl,
                        )

            def out_proj(st, oc):
                ps = ps_pool.tile([P, 512], f32, tag="ps")
                for et in range(NJ):
                    nc.tensor.matmul(
                        ps,
                        lhsT=c_sb[et][:, st * P:(st + 1) * P],
                        rhs=wo_sb[:, et, oc * 512:(oc + 1) * 512],
                        start=(et == 0),
                        stop=(et == NJ - 1),
                    )
                ot = outp.tile([P, 512], f32, tag="ot")
                nc.scalar.copy(ot, ps)
                nc.sync.dma_start(
                    out=out_d[st * P:(st + 1) * P, oc * 512:(oc + 1) * 512],
                    in_=ot,
                )

            # ---- emission order: interleave so V-proj / next j-tile's
            # projections (PE work) can fill the ACT-bound exp stretches ----
            # pipeline: pair 0's scores first, then all of V, then each
            # subsequent pair's scores followed by the previous pair's AV --
            # so pt/rl pool slots (bufs=2) recycle without stalling the flow
            pt_pairs = []
            rl_pairs = []

            def scores_block(jt):
                proj_qk(jt, xk_sb, kt_sb[jt], None)
                proj_qk(jt, xq_sb, qt_sb[jt], bq_sb[:, jt:jt + 1])
                pair = [pt_pool.tile([P, NE, S], bf16, tag="pt", name=f"pt{hh}")
                        for hh in range(2)]
                pt_pairs.append(pair)
                scores_exp(jt, pair)
                if av_mode == "ones":
                    rl_pairs.append(None)
                elif jt == NJ - 1:
                    rl_pairs.append(None)  # handled inside av_last
                else:
                    rl_pairs.append([denom(jt, hh, pair[hh]) for hh in range(2)])

            scores_block(0)
            for st in range(NST):
                proj_v(st)
            def do_av(jt):
                if av_mode == "ones":
                    av_ones(jt, pt_pairs[jt])
                elif jt == NJ - 1:
                    av_last(jt, pt_pairs[jt])
                else:
                    av_pair(jt, pt_pairs[jt], rl_pairs[jt])

            for jt in range(1, NJ):
                scores_block(jt)
                do_av(jt - 1)
            do_av(NJ - 1)
            for st in range(NST):
                for oc in range(NQ):
                    out_proj(st, oc)

        for _ in range(reps):
            body()

    nc.finalize()
    return nc


def _get_nc(reps=1, av_mode="packed"):
    key = ("nc", reps, av_mode)
    if key not in _CACHE:
        _CACHE[key] = _build_program(reps, av_mode)
    return _CACHE[key]


def make_in_maps(queries, keys, values, Wq_w, Wq_b, Wo_w, Wo_b):
    in_maps = []
    for c in range(NCORES):
        b, g = c // 2, c % 2
        js = slice(g * EH, (g + 1) * EH)
        in_maps.append({
            "xq_t": np.ascontiguousarray(queries[b].T).astype(BF16),
            "xk_t": np.ascontiguousarray(keys[b].T).astype(BF16),
            "xv_t": np.ascontiguousarray(values[b].T).astype(BF16),
            "wq_t": np.ascontiguousarray(Wq_w[js, :].T).astype(BF16),
            "wo_t": np.ascontiguousarray(Wo_w[:, js].T).astype(BF16),
            "bq": np.ascontiguousarray(Wq_b[js].reshape(NJ, P).T),
        })
    return in_maps


def assemble_output(results, Wq_b, Wo_w, Wo_b):
    # host-side unshard: sum the two head-group partials per batch, add the
    # folded bias (Wo_b + V-bias routed through Wo since attn rows sum to 1)
    bias_total = (Wo_w @ Wq_b + Wo_b).astype(np.float32)
    out = np.empty((B, S, E), np.float32)
    for b in range(B):
        out[b] = results[2 * b]["out_partial"] + results[2 * b + 1]["out_partial"]
    out += bias_total
    return out


def kernel(queries, keys, values, Wq_w, Wq_b, Wo_w, Wo_b, num_heads):
    from concourse.bass_utils import run_bass_kernel_spmd

    queries = np.asarray(queries, np.float32)
    keys = np.asarray(keys, np.float32)
    values = np.asarray(values, np.float32)
    Wq_w = np.asarray(Wq_w, np.float32)
    Wq_b = np.asarray(Wq_b, np.float32)
    Wo_w = np.asarray(Wo_w, np.float32)
    Wo_b = np.asarray(Wo_b, np.float32)
    assert int(num_heads) == H

    nc = _get_nc()
    in_maps = make_in_maps(queries, keys, values, Wq_w, Wq_b, Wo_w, Wo_b)
    res = run_bass_kernel_spmd(nc, in_maps, core_ids=list(range(NCORES)))
    _CACHE["last_results"] = res
    return assemble_output(res.results, Wq_b, Wo_w, Wo_b)
